# revision 7
# baseline (speedup 1.0000x reference)
"""Trainium2 Bass kernel for nn_ComputeCorr (retrieval_knn), v2.

Math (per batch pair b, D=64):
  d[m,n] = ||tf_m - sf_n||^2   (tf = tgt_f[b].T rows, sf = src_f[b].T rows)
  src_corr[n] = sum_m W[m,n] [tgt|1][m,:] / (...)    W = softmax over m
  tgt_corr[m] = sum_n W[m,n] [src|1][n,:] / (...)    softmax over n

Key restructure vs v1: the score matrix is exp'd ONCE per element
(U = exp(2ab - aa - bb + c0), bf16) and consumed by BOTH softmax
directions, halving the ScalarE exp work that co-bottlenecked v1:
  - per-n shift (-aa/2) rides one fp16 K-row (K=65); its fp16 rounding
    residual cancels for the src side and is folded into the tgt-side
    weights on the host.
  - per-m shift (c0 - bb) is the ACT per-partition fp32 bias: exact.
  - src side: col-tiled (tile_position) 4-slot PV matmuls stream U
    directly: out[4-row slot, 512 n] per m-chunk, one PSUM bank.
  - tgt side: each m-chunk's full U row [128, 4096] is transposed by
    the DMA xbar engine (dma_start_transpose: 14ns/16x128 tile, zero
    PE/ACT/DVE cost) into a [128, 32, 128] tile; col-tiled PV matmuls
    stream its 128-col slices, accumulating all 32 n-chunks. The xbar
    row interleave (row p, slab k <-> n = p*32+k) is mirrored in the
    host-side packing of the [src|1] weights - contractions don't care
    about n order.
  - Outputs leave as per-block PSUM partials (DVE bank copy -> DMA);
    the host does the final partial sums + divisions (B x 4 x 4096).

Sharding: 8 cores = 4 batches x 2 m-halves. Core (b,h) owns m rows
[h*2048,(h+1)*2048): it computes src-numerator partials over its
m-rows for ALL n (host adds the two cores) and complete tgt_corr
partials for its m-half.
"""

import os
import sys

import numpy as np

for _p in ("/opt/trn_rl_repo", "/root/.axon_site/_ro/trn_rl_repo"):
    if os.path.isdir(_p) and _p not in sys.path:
        sys.path.insert(0, _p)

import ml_dtypes

import concourse.bacc as bacc
import concourse.tile as tile
from concourse import mybir
from concourse.bass_utils import run_bass_kernel_spmd

B, N, D = 4, 4096, 64
MH = 2048  # m rows per core
NCORES = 8
C0 = 80.0
K = D + 1  # features + n-shift row
NB2 = 1024  # exp tile width (2 PSUM banks)
NMC = MH // 128  # 16 m-chunks per core
NMB = 4  # m-chunk groups of 4
NNB2 = N // NB2  # 4 n superblocks
F32 = mybir.dt.float32
F16 = mybir.dt.float16
BF16 = mybir.dt.bfloat16
NPBF = ml_dtypes.bfloat16

_PROG = None


def _build():
    nc = bacc.Bacc("TRN2", target_bir_lowering=False, debug=False)

    lhs_d = nc.dram_tensor("lhs", [K, MH], F16, kind="ExternalInput").ap()
    rhs_d = nc.dram_tensor("rhs", [K, N], F16, kind="ExternalInput").ap()
    bias_d = nc.dram_tensor("bias", [128, NMC], F32, kind="ExternalInput").ap()
    vt_d = nc.dram_tensor("vt", [128, NMC * 4], F16, kind="ExternalInput").ap()
    vs_d = nc.dram_tensor("vs", [128, 32 * 4], BF16, kind="ExternalInput").ap()
    wsrc_d = nc.dram_tensor(
        "wsrc", [NMB, NNB2, 16, NB2], F32, kind="ExternalOutput"
    ).ap()
    wtgt_d = nc.dram_tensor("wtgt", [NMB, 16, 512], F32, kind="ExternalOutput").ap()

    with tile.TileContext(nc) as tc:
        with (
            tc.tile_pool(name="inp", bufs=1) as inp,
            tc.tile_pool(name="upool", bufs=9) as upool,
            tc.tile_pool(name="utpool", bufs=3) as utpool,
            tc.tile_pool(name="osb", bufs=2) as osb,
            tc.tile_pool(name="spool", bufs=2, space="PSUM") as spool,
            tc.tile_pool(name="wspool", bufs=1, space="PSUM") as wspool,
            tc.tile_pool(name="wtpool", bufs=2, space="PSUM") as wtpool,
        ):
            lhs = inp.tile([K, MH], F16, tag="lhs", name="lhs")
            rhs = inp.tile([K, N], F16, tag="rhs", name="rhs")
            bias = inp.tile([128, NMC], F32, tag="bias", name="bias")
            vt = inp.tile([128, NMC * 4], F16, tag="vt", name="vt")
            vs = inp.tile([128, 32 * 4], BF16, tag="vs", name="vs")

            # Inputs: gpsimd SWDGE ring (25ns triggers) for everything
            # except the first rhs superblock, which rides sync so the
            # first matmul unblocks ASAP. Consumption order.
            nc.sync.dma_start(out=rhs[:, :NB2], in_=rhs_d[:, :NB2])
            nc.gpsimd.dma_start(out=bias, in_=bias_d)
            nc.gpsimd.dma_start(out=lhs[:, :512], in_=lhs_d[:, :512])
            nc.gpsimd.dma_start(out=vt, in_=vt_d)
            nc.gpsimd.dma_start(out=vs, in_=vs_d)
            nc.sync.dma_start(out=rhs[:, NB2 : 2 * NB2], in_=rhs_d[:, NB2 : 2 * NB2])
            nc.gpsimd.dma_start(out=lhs[:, 512:], in_=lhs_d[:, 512:])
            nc.sync.dma_start(out=rhs[:, 2 * NB2 :], in_=rhs_d[:, 2 * NB2 :])

            # Deferred-thunk software pipeline: PV work trails its data
            # producers by ~2 steps so the PE never waits on the xbar
            # transpose or ACT.
            pend = []

            def flush(n_keep):
                while len(pend) > n_keep:
                    pend.pop(0)()

            for mb in range(NMB):
                wt = wtpool.tile([128, 512], F32, tag="wt", name="wt")
                us = [None] * 4
                for j in range(4):
                    mc = mb * 4 + j
                    u = upool.tile([128, N], BF16, tag="u", name="u")
                    us[j] = u
                    for nb2 in range(NNB2):
                        s = spool.tile([128, NB2], F32, tag="s", name="s")
                        for half in range(2):
                            nc.tensor.matmul(
                                s[:, half * 512 : (half + 1) * 512],
                                lhsT=lhs[:, mc * 128 : (mc + 1) * 128],
                                rhs=rhs[:, nb2 * NB2 + half * 512 :][:, :512],
                                start=True,
                                stop=True,
                            )
                        nc.scalar.activation(
                            out=u[:, nb2 * NB2 : (nb2 + 1) * NB2],
                            in_=s,
                            func=mybir.ActivationFunctionType.Exp,
                            scale=2.0,
                            bias=bias[:, mc : mc + 1],
                        )
                        flush(2)
                    ut = utpool.tile([128, 32, 128], BF16, tag="ut", name="ut")
                    ring = nc.sync if mc % 2 == 0 else nc.scalar
                    ring.dma_start_transpose(out=ut, in_=u)

                    def pvtgt(ut=ut, j=j, wt=wt):
                        for r in range(8):
                            for c in range(4):
                                k = r * 4 + c
                                nc.tensor.matmul(
                                    wt[32 * c : 32 * c + 4][
                                        :, j * 128 : (j + 1) * 128
                                    ],
                                    lhsT=vs[:, 4 * k : 4 * k + 4],
                                    rhs=ut[:, k, :],
                                    start=(r == 0),
                                    stop=(r == 7),
                                    tile_position=(0, 32 * c),
                                    skip_group_check=True,
                                )

                    pend.append(pvtgt)

                def group_epilogue(us=us, wt=wt, mb=mb):
                    for nb2 in range(NNB2):
                        ws = wspool.tile([128, NB2], F32, tag="ws", name="ws")
                        for half in range(2):
                            for c in range(4):
                                nc.tensor.matmul(
                                    ws[32 * c : 32 * c + 4][
                                        :, half * 512 : (half + 1) * 512
                                    ],
                                    lhsT=vt[:, (mb * 4 + c) * 4 :][:, :4],
                                    rhs=us[c][
                                        :,
                                        nb2 * NB2 + half * 512 :,
                                    ][:, :512],
                                    start=True,
                                    stop=True,
                                    tile_position=(0, 32 * c),
                                    skip_group_check=True,
                                )
                        ws_sb = osb.tile([128, NB2], F32, tag="ws_sb", name="ws_sb")
                        nc.vector.tensor_copy(ws_sb, ws)
                        for c in range(4):
                            nc.gpsimd.dma_start(
                                out=wsrc_d[mb, nb2, 4 * c : 4 * c + 4, :],
                                in_=ws_sb[32 * c : 32 * c + 4, :],
                            )
                    wt_sb = osb.tile([128, 512], F32, tag="wt_sb", name="wt_sb")
                    nc.vector.tensor_copy(wt_sb, wt)
                    for c in range(4):
                        nc.gpsimd.dma_start(
                            out=wtgt_d[mb, 4 * c : 4 * c + 4, :],
                            in_=wt_sb[32 * c : 32 * c + 4, :],
                        )

                pend.append(group_epilogue)
            flush(0)

    nc.compile()
    return nc


def _prep_inputs(src, tgt, src_f, tgt_f):
    src = np.ascontiguousarray(src, dtype=np.float32)
    tgt = np.ascontiguousarray(tgt, dtype=np.float32)
    src_f = np.ascontiguousarray(src_f, dtype=np.float32)
    tgt_f = np.ascontiguousarray(tgt_f, dtype=np.float32)

    in_maps = []
    for core in range(NCORES):
        b, h = divmod(core, 2)
        msl = slice(h * MH, (h + 1) * MH)
        L = tgt_f[b][:, msl]  # [64, MH]
        R = src_f[b]  # [64, N]
        aa = (R * R).sum(axis=0)  # [N]
        bb = (L * L).sum(axis=0)  # [MH]
        s_n = (-0.5 * aa).astype(np.float16)  # fp16 shift row
        r_n = s_n.astype(np.float32) + 0.5 * aa  # rounding residual
        lhs = np.vstack([L.astype(np.float16), np.ones((1, MH), np.float16)])
        rhs = np.vstack([R.astype(np.float16), s_n[None, :]])
        bias = np.ascontiguousarray(
            (C0 - bb).reshape(NMC, 128).T.astype(np.float32)
        )  # [128, 16]
        # vt: col block mc = [tgt|1] rows of that m-chunk
        vtgt = np.concatenate([tgt[b][msl], np.ones((MH, 1), np.float32)], 1)
        vt = np.ascontiguousarray(
            vtgt.reshape(NMC, 128, 4).transpose(1, 0, 2).reshape(128, NMC * 4)
        ).astype(np.float16)
        # vs: xbar-permuted [src|1]*exp(-2 r_n): slab k, row p <-> n=k*128+p
        vsrc = np.concatenate([src[b], np.ones((N, 1), np.float32)], 1)
        vsrc = vsrc * np.exp(-2.0 * r_n)[:, None]
        vs = np.ascontiguousarray(
            vsrc.reshape(32, 128, 4).transpose(1, 0, 2).reshape(128, 128)
        )
        in_maps.append(
            {
                "lhs": np.ascontiguousarray(lhs),
                "rhs": np.ascontiguousarray(rhs),
                "bias": bias,
                "vt": vt,
                "vs": vs.astype(NPBF),
            }
        )
    return in_maps


def _postprocess(results):
    src_corr = np.zeros((B, N, 3), np.float32)
    tgt_corr = np.zeros((B, N, 3), np.float32)
    for b in range(B):
        wsum = None
        for h in range(2):
            w = results[2 * b + h]["wsrc"].astype(np.float64)  # [mb,nb2,16,1024]
            w = w.reshape(NMB, NNB2, 4, 4, NB2).sum(axis=(0, 2))  # [nb2,4,1024]
            w = w.transpose(0, 2, 1).reshape(N, 4)  # [n, 4]
            wsum = w if wsum is None else wsum + w
        src_corr[b] = (wsum[:, :3] / wsum[:, 3:4]).astype(np.float32)
        for h in range(2):
            w = results[2 * b + h]["wtgt"].astype(np.float64)  # [mb,16,512]
            w = w.reshape(NMB, 4, 4, 512).sum(axis=1)  # [mb, 4, 512]
            w = w.transpose(0, 2, 1).reshape(MH, 4)  # [m', 4]
            tgt_corr[b, h * MH : (h + 1) * MH] = (w[:, :3] / w[:, 3:4]).astype(
                np.float32
            )
    return src_corr, tgt_corr


def run(inputs, trace=False, **kw):
    global _PROG
    if _PROG is None:
        _PROG = _build()
    in_maps = _prep_inputs(
        inputs["src"], inputs["tgt"], inputs["src_f"], inputs["tgt_f"]
    )
    bkr = run_bass_kernel_spmd(
        _PROG, in_maps, core_ids=list(range(NCORES)), trace=trace, **kw
    )
    return _postprocess(bkr.results), bkr


def kernel(**inputs):
    out, _ = run(inputs)
    return out


# revision 8
# speedup vs baseline: 1.4519x; 1.4519x over previous
"""Trainium2 Bass kernel for nn_ComputeCorr (retrieval_knn), v2.

Math (per batch pair b, D=64):
  d[m,n] = ||tf_m - sf_n||^2   (tf = tgt_f[b].T rows, sf = src_f[b].T rows)
  src_corr[n] = sum_m W[m,n] [tgt|1][m,:] / (...)    W = softmax over m
  tgt_corr[m] = sum_n W[m,n] [src|1][n,:] / (...)    softmax over n

Key restructure vs v1: the score matrix is exp'd ONCE per element
(U = exp(2ab - aa - bb + c0), bf16) and consumed by BOTH softmax
directions, halving the ScalarE exp work that co-bottlenecked v1:
  - per-n shift (-aa/2) rides one fp16 K-row (K=65); its fp16 rounding
    residual cancels for the src side and is folded into the tgt-side
    weights on the host.
  - per-m shift (c0 - bb) is the ACT per-partition fp32 bias: exact.
  - src side: col-tiled (tile_position) 4-slot PV matmuls stream U
    directly: out[4-row slot, 512 n] per m-chunk, one PSUM bank.
  - tgt side: each m-chunk's full U row [128, 4096] is transposed by
    the DMA xbar engine (dma_start_transpose: 14ns/16x128 tile, zero
    PE/ACT/DVE cost) into a [128, 32, 128] tile; col-tiled PV matmuls
    stream its 128-col slices, accumulating all 32 n-chunks. The xbar
    row interleave (row p, slab k <-> n = p*32+k) is mirrored in the
    host-side packing of the [src|1] weights - contractions don't care
    about n order.
  - Outputs leave as per-block PSUM partials (DVE bank copy -> DMA);
    the host does the final partial sums + divisions (B x 4 x 4096).

Sharding: 8 cores = 4 batches x 2 m-halves. Core (b,h) owns m rows
[h*2048,(h+1)*2048): it computes src-numerator partials over its
m-rows for ALL n (host adds the two cores) and complete tgt_corr
partials for its m-half.
"""

import os
import sys

import numpy as np

for _p in ("/opt/trn_rl_repo", "/root/.axon_site/_ro/trn_rl_repo"):
    if os.path.isdir(_p) and _p not in sys.path:
        sys.path.insert(0, _p)

import ml_dtypes

import concourse.bacc as bacc
import concourse.tile as tile
from concourse import mybir
from concourse.bass_utils import run_bass_kernel_spmd

B, N, D = 4, 4096, 64
MH = 2048  # m rows per core
NCORES = 8
C0 = 80.0
K = D + 1  # features + n-shift row
NB2 = 1024  # exp tile width (2 PSUM banks)
NMC = MH // 128  # 16 m-chunks per core
NMB = 4  # m-chunk groups of 4
NNB2 = N // NB2  # 4 n superblocks
F32 = mybir.dt.float32
F16 = mybir.dt.float16
BF16 = mybir.dt.bfloat16
NPBF = ml_dtypes.bfloat16

_PROG = None


def _build():
    nc = bacc.Bacc("TRN2", target_bir_lowering=False, debug=False)

    lhs_d = nc.dram_tensor("lhs", [K, MH], F16, kind="ExternalInput").ap()
    rhs_d = nc.dram_tensor("rhs", [K, N], F16, kind="ExternalInput").ap()
    bias_d = nc.dram_tensor("bias", [128, NMC], F32, kind="ExternalInput").ap()
    vt_d = nc.dram_tensor("vt", [128, NMC * 4], F16, kind="ExternalInput").ap()
    vs_d = nc.dram_tensor("vs", [128, 32 * 4], BF16, kind="ExternalInput").ap()
    wsrc_d = nc.dram_tensor(
        "wsrc", [NNB2, 16, NB2], F32, kind="ExternalOutput"
    ).ap()
    wtgt_d = nc.dram_tensor("wtgt", [NMB, 16, 512], F32, kind="ExternalOutput").ap()

    with tile.TileContext(nc) as tc:
        with (
            tc.tile_pool(name="inp", bufs=1) as inp,
            tc.tile_pool(name="upool", bufs=9) as upool,
            tc.tile_pool(name="utpool", bufs=3) as utpool,
            tc.tile_pool(name="osb", bufs=2) as osb,
            tc.tile_pool(name="spool", bufs=2, space="PSUM") as spool,
            tc.tile_pool(name="wspool", bufs=1, space="PSUM") as wspool,
            tc.tile_pool(name="wtpool", bufs=2, space="PSUM") as wtpool,
        ):
            lhs = inp.tile([K, MH], F16, tag="lhs", name="lhs")
            rhs = inp.tile([K, N], F16, tag="rhs", name="rhs")
            bias = inp.tile([128, NMC], F32, tag="bias", name="bias")
            vt = inp.tile([128, NMC * 4], F16, tag="vt", name="vt")
            vs = inp.tile([128, 32 * 4], BF16, tag="vs", name="vs")
            ws_acc = inp.tile([128, N], F32, tag="ws_acc", name="ws_acc")

            # Inputs split so the first m-chunk's scores (which sweep ALL
            # of rhs) unblock fast: rhs superblocks alternate sync/gpsimd;
            # everything else rides gpsimd in consumption order. The sync
            # ring is otherwise reserved for the 16 xbar transposes.
            nc.sync.dma_start(out=rhs[:, :NB2], in_=rhs_d[:, :NB2])
            nc.gpsimd.dma_start(out=bias, in_=bias_d)
            nc.gpsimd.dma_start(out=rhs[:, NB2 : 2 * NB2], in_=rhs_d[:, NB2 : 2 * NB2])
            nc.sync.dma_start(out=rhs[:, 2 * NB2 : 3 * NB2], in_=rhs_d[:, 2 * NB2 : 3 * NB2])
            nc.gpsimd.dma_start(out=lhs[:, :512], in_=lhs_d[:, :512])
            nc.gpsimd.dma_start(out=rhs[:, 3 * NB2 :], in_=rhs_d[:, 3 * NB2 :])
            nc.gpsimd.dma_start(out=vs, in_=vs_d)
            nc.gpsimd.dma_start(out=vt, in_=vt_d)
            nc.gpsimd.dma_start(out=lhs[:, 512:], in_=lhs_d[:, 512:])

            # Deferred-thunk software pipeline: PV work trails its data
            # producers by ~2 steps so the PE never waits on the xbar
            # transpose or ACT.
            pend = []

            def flush(n_keep):
                while len(pend) > n_keep:
                    pend.pop(0)()

            for mb in range(NMB):
                wt = wtpool.tile([128, 512], F32, tag="wt", name="wt")
                us = [None] * 4
                for j in range(4):
                    mc = mb * 4 + j
                    u = upool.tile([128, N], BF16, tag="u", name="u")
                    us[j] = u
                    for nb2 in range(NNB2):
                        s = spool.tile([128, NB2], F32, tag="s", name="s")
                        for half in range(2):
                            nc.tensor.matmul(
                                s[:, half * 512 : (half + 1) * 512],
                                lhsT=lhs[:, mc * 128 : (mc + 1) * 128],
                                rhs=rhs[:, nb2 * NB2 + half * 512 :][:, :512],
                                start=True,
                                stop=True,
                            )
                        nc.scalar.activation(
                            out=u[:, nb2 * NB2 : (nb2 + 1) * NB2],
                            in_=s,
                            func=mybir.ActivationFunctionType.Exp,
                            scale=2.0,
                            bias=bias[:, mc : mc + 1],
                        )
                        flush(2)
                    ut = utpool.tile([128, 32, 128], BF16, tag="ut", name="ut")
                    ring = nc.scalar if mc % 4 == 3 else nc.sync
                    ring.dma_start_transpose(out=ut, in_=u)

                    def pvtgt(ut=ut, j=j, wt=wt):
                        for r in range(8):
                            for c in range(4):
                                k = r * 4 + c
                                nc.tensor.matmul(
                                    wt[32 * c : 32 * c + 4][
                                        :, j * 128 : (j + 1) * 128
                                    ],
                                    lhsT=vs[:, 4 * k : 4 * k + 4],
                                    rhs=ut[:, k, :],
                                    start=(r == 0),
                                    stop=(r == 7),
                                    tile_position=(0, 32 * c),
                                    skip_group_check=True,
                                )

                    pend.append(pvtgt)

                def group_epilogue(us=us, wt=wt, mb=mb):
                    for nb2 in range(NNB2):
                        ws = wspool.tile([128, NB2], F32, tag="ws", name="ws")
                        for half in range(2):
                            for c in range(4):
                                nc.tensor.matmul(
                                    ws[32 * c : 32 * c + 4][
                                        :, half * 512 : (half + 1) * 512
                                    ],
                                    lhsT=vt[:, (mb * 4 + c) * 4 :][:, :4],
                                    rhs=us[c][
                                        :,
                                        nb2 * NB2 + half * 512 :,
                                    ][:, :512],
                                    start=True,
                                    stop=True,
                                    tile_position=(0, 32 * c),
                                    skip_group_check=True,
                                )
                        acc = ws_acc[:, nb2 * NB2 : (nb2 + 1) * NB2]
                        if mb == 0:
                            nc.vector.tensor_copy(acc, ws)
                        else:
                            nc.vector.tensor_add(acc, acc, ws)
                        if mb == NMB - 1:
                            for c in range(4):
                                nc.gpsimd.dma_start(
                                    out=wsrc_d[nb2, 4 * c : 4 * c + 4, :],
                                    in_=acc[32 * c : 32 * c + 4, :],
                                )
                    wt_sb = osb.tile([128, 512], F32, tag="wt_sb", name="wt_sb")
                    nc.vector.tensor_copy(wt_sb, wt)
                    for c in range(4):
                        nc.gpsimd.dma_start(
                            out=wtgt_d[mb, 4 * c : 4 * c + 4, :],
                            in_=wt_sb[32 * c : 32 * c + 4, :],
                        )

                pend.append(group_epilogue)
            flush(0)

    nc.compile()
    return nc


def _prep_inputs(src, tgt, src_f, tgt_f):
    src = np.ascontiguousarray(src, dtype=np.float32)
    tgt = np.ascontiguousarray(tgt, dtype=np.float32)
    src_f = np.ascontiguousarray(src_f, dtype=np.float32)
    tgt_f = np.ascontiguousarray(tgt_f, dtype=np.float32)

    in_maps = []
    for core in range(NCORES):
        b, h = divmod(core, 2)
        msl = slice(h * MH, (h + 1) * MH)
        L = tgt_f[b][:, msl]  # [64, MH]
        R = src_f[b]  # [64, N]
        aa = (R * R).sum(axis=0)  # [N]
        bb = (L * L).sum(axis=0)  # [MH]
        s_n = (-0.5 * aa).astype(np.float16)  # fp16 shift row
        r_n = s_n.astype(np.float32) + 0.5 * aa  # rounding residual
        lhs = np.vstack([L.astype(np.float16), np.ones((1, MH), np.float16)])
        rhs = np.vstack([R.astype(np.float16), s_n[None, :]])
        bias = np.ascontiguousarray(
            (C0 - bb).reshape(NMC, 128).T.astype(np.float32)
        )  # [128, 16]
        # vt: col block mc = [tgt|1] rows of that m-chunk
        vtgt = np.concatenate([tgt[b][msl], np.ones((MH, 1), np.float32)], 1)
        vt = np.ascontiguousarray(
            vtgt.reshape(NMC, 128, 4).transpose(1, 0, 2).reshape(128, NMC * 4)
        ).astype(np.float16)
        # vs: xbar-permuted [src|1]*exp(-2 r_n): slab k, row p <-> n=k*128+p
        vsrc = np.concatenate([src[b], np.ones((N, 1), np.float32)], 1)
        vsrc = vsrc * np.exp(-2.0 * r_n)[:, None]
        vs = np.ascontiguousarray(
            vsrc.reshape(32, 128, 4).transpose(1, 0, 2).reshape(128, 128)
        )
        in_maps.append(
            {
                "lhs": np.ascontiguousarray(lhs),
                "rhs": np.ascontiguousarray(rhs),
                "bias": bias,
                "vt": vt,
                "vs": vs.astype(NPBF),
            }
        )
    return in_maps


def _postprocess(results):
    src_corr = np.zeros((B, N, 3), np.float32)
    tgt_corr = np.zeros((B, N, 3), np.float32)
    for b in range(B):
        wsum = None
        for h in range(2):
            w = results[2 * b + h]["wsrc"].astype(np.float64)  # [nb2,16,1024]
            w = w.reshape(NNB2, 4, 4, NB2).sum(axis=1)  # [nb2,4,1024]
            w = w.transpose(0, 2, 1).reshape(N, 4)  # [n, 4]
            wsum = w if wsum is None else wsum + w
        src_corr[b] = (wsum[:, :3] / wsum[:, 3:4]).astype(np.float32)
        for h in range(2):
            w = results[2 * b + h]["wtgt"].astype(np.float64)  # [mb,16,512]
            w = w.reshape(NMB, 4, 4, 512).sum(axis=1)  # [mb, 4, 512]
            w = w.transpose(0, 2, 1).reshape(MH, 4)  # [m', 4]
            tgt_corr[b, h * MH : (h + 1) * MH] = (w[:, :3] / w[:, 3:4]).astype(
                np.float32
            )
    return src_corr, tgt_corr


def run(inputs, trace=False, **kw):
    global _PROG
    if _PROG is None:
        _PROG = _build()
    in_maps = _prep_inputs(
        inputs["src"], inputs["tgt"], inputs["src_f"], inputs["tgt_f"]
    )
    bkr = run_bass_kernel_spmd(
        _PROG, in_maps, core_ids=list(range(NCORES)), trace=trace, **kw
    )
    return _postprocess(bkr.results), bkr


def kernel(**inputs):
    out, _ = run(inputs)
    return out


# revision 10
# speedup vs baseline: 1.5379x; 1.0592x over previous
"""Trainium2 Bass kernel for nn_ComputeCorr (retrieval_knn), v2.

Math (per batch pair b, D=64):
  d[m,n] = ||tf_m - sf_n||^2   (tf = tgt_f[b].T rows, sf = src_f[b].T rows)
  src_corr[n] = sum_m W[m,n] [tgt|1][m,:] / (...)    W = softmax over m
  tgt_corr[m] = sum_n W[m,n] [src|1][n,:] / (...)    softmax over n

Key restructure vs v1: the score matrix is exp'd ONCE per element
(U = exp(2ab - aa - bb + c0), bf16) and consumed by BOTH softmax
directions, halving the ScalarE exp work that co-bottlenecked v1:
  - per-n shift (-aa/2) rides one fp16 K-row (K=65); its fp16 rounding
    residual cancels for the src side and is folded into the tgt-side
    weights on the host.
  - per-m shift (c0 - bb) is the ACT per-partition fp32 bias: exact.
  - src side: col-tiled (tile_position) 4-slot PV matmuls stream U
    directly: out[4-row slot, 512 n] per m-chunk, one PSUM bank.
  - tgt side: each m-chunk's full U row [128, 4096] is transposed by
    the DMA xbar engine (dma_start_transpose: 14ns/16x128 tile, zero
    PE/ACT/DVE cost) into a [128, 32, 128] tile; col-tiled PV matmuls
    stream its 128-col slices, accumulating all 32 n-chunks. The xbar
    row interleave (row p, slab k <-> n = p*32+k) is mirrored in the
    host-side packing of the [src|1] weights - contractions don't care
    about n order.
  - Outputs leave as per-block PSUM partials (DVE bank copy -> DMA);
    the host does the final partial sums + divisions (B x 4 x 4096).

Sharding: 8 cores = 4 batches x 2 m-halves. Core (b,h) owns m rows
[h*2048,(h+1)*2048): it computes src-numerator partials over its
m-rows for ALL n (host adds the two cores) and complete tgt_corr
partials for its m-half.
"""

import os
import sys

import numpy as np

for _p in ("/opt/trn_rl_repo", "/root/.axon_site/_ro/trn_rl_repo"):
    if os.path.isdir(_p) and _p not in sys.path:
        sys.path.insert(0, _p)

import ml_dtypes

import concourse.bacc as bacc
import concourse.tile as tile
from concourse import mybir
from concourse.bass_utils import run_bass_kernel_spmd

B, N, D = 4, 4096, 64
MH = 2048  # m rows per core
NCORES = 8
C0 = 80.0
K = D + 1  # features + n-shift row
NB2 = 1024  # exp tile width (2 PSUM banks)
NMC = MH // 128  # 16 m-chunks per core
NMB = 4  # m-chunk groups of 4
NNB2 = N // NB2  # 4 n superblocks
F32 = mybir.dt.float32
F16 = mybir.dt.float16
BF16 = mybir.dt.bfloat16
NPBF = ml_dtypes.bfloat16

_PROG = None


def _build():
    nc = bacc.Bacc("TRN2", target_bir_lowering=False, debug=False)

    lhs_d = nc.dram_tensor("lhs", [K, MH], F16, kind="ExternalInput").ap()
    rhs_d = nc.dram_tensor("rhs", [K, N], F16, kind="ExternalInput").ap()
    bias_d = nc.dram_tensor("bias", [128, NMC], F32, kind="ExternalInput").ap()
    vt_d = nc.dram_tensor("vt", [128, NMC * 4], F16, kind="ExternalInput").ap()
    vs_d = nc.dram_tensor("vs", [128, 32 * 4], BF16, kind="ExternalInput").ap()
    wsrc_d = nc.dram_tensor(
        "wsrc", [NNB2, 16, NB2], F32, kind="ExternalOutput"
    ).ap()
    wtgt_d = nc.dram_tensor("wtgt", [NMB, 16, 512], F32, kind="ExternalOutput").ap()

    with tile.TileContext(nc) as tc:
        with (
            tc.tile_pool(name="inp", bufs=1) as inp,
            tc.tile_pool(name="upool", bufs=9) as upool,
            tc.tile_pool(name="utpool", bufs=3) as utpool,
            tc.tile_pool(name="osb", bufs=2) as osb,
            tc.tile_pool(name="spool", bufs=2, space="PSUM") as spool,
            tc.tile_pool(name="wspool", bufs=1, space="PSUM") as wspool,
            tc.tile_pool(name="wtpool", bufs=2, space="PSUM") as wtpool,
        ):
            lhs = inp.tile([K, MH], F16, tag="lhs", name="lhs")
            rhs = inp.tile([K, N], F16, tag="rhs", name="rhs")
            bias = inp.tile([128, NMC], F32, tag="bias", name="bias")
            vt = inp.tile([128, NMC * 4], F16, tag="vt", name="vt")
            vs = inp.tile([128, 32 * 4], BF16, tag="vs", name="vs")
            ws_acc = inp.tile([128, N], F32, tag="ws_acc", name="ws_acc")

            # Inputs split so the first m-chunk's scores (which sweep ALL
            # of rhs) unblock fast: rhs superblocks alternate sync/gpsimd;
            # everything else rides gpsimd in consumption order. The sync
            # ring is otherwise reserved for the 16 xbar transposes.
            nc.sync.dma_start(out=rhs[:, :NB2], in_=rhs_d[:, :NB2])
            nc.gpsimd.dma_start(out=bias, in_=bias_d)
            nc.gpsimd.dma_start(out=rhs[:, NB2 : 2 * NB2], in_=rhs_d[:, NB2 : 2 * NB2])
            nc.sync.dma_start(out=rhs[:, 2 * NB2 : 3 * NB2], in_=rhs_d[:, 2 * NB2 : 3 * NB2])
            nc.gpsimd.dma_start(out=lhs[:, :512], in_=lhs_d[:, :512])
            nc.gpsimd.dma_start(out=rhs[:, 3 * NB2 :], in_=rhs_d[:, 3 * NB2 :])
            nc.gpsimd.dma_start(out=vs, in_=vs_d)
            nc.gpsimd.dma_start(out=vt, in_=vt_d)
            nc.gpsimd.dma_start(out=lhs[:, 512:], in_=lhs_d[:, 512:])

            # Deferred-thunk software pipeline: PV work trails its data
            # producers by ~2 steps so the PE never waits on the xbar
            # transpose or ACT.
            pend = []

            def flush(n_keep):
                while len(pend) > n_keep:
                    pend.pop(0)()

            for mb in range(NMB):
                wt = wtpool.tile([128, 512], F32, tag="wt", name="wt")
                us = [None] * 4
                for j in range(4):
                    mc = mb * 4 + j
                    u = upool.tile([128, N], BF16, tag="u", name="u")
                    us[j] = u
                    for nb2 in range(NNB2):
                        s = spool.tile([128, NB2], F32, tag="s", name="s")
                        for half in range(2):
                            nc.tensor.matmul(
                                s[:, half * 512 : (half + 1) * 512],
                                lhsT=lhs[:, mc * 128 : (mc + 1) * 128],
                                rhs=rhs[:, nb2 * NB2 + half * 512 :][:, :512],
                                start=True,
                                stop=True,
                            )
                        nc.scalar.activation(
                            out=u[:, nb2 * NB2 : (nb2 + 1) * NB2],
                            in_=s,
                            func=mybir.ActivationFunctionType.Exp,
                            scale=2.0,
                            bias=bias[:, mc : mc + 1],
                        )
                        flush(2)
                    ut = utpool.tile([128, 32, 128], BF16, tag="ut", name="ut")
                    ring = nc.scalar if mc % 4 == 3 else nc.sync
                    ring.dma_start_transpose(out=ut, in_=u)

                    def pvtgt(ut=ut, j=j, wt=wt):
                        for r in range(8):
                            for c in range(4):
                                k = r * 4 + c
                                nc.tensor.matmul(
                                    wt[32 * c : 32 * c + 4][
                                        :, j * 128 : (j + 1) * 128
                                    ],
                                    lhsT=vs[:, 4 * k : 4 * k + 4],
                                    rhs=ut[:, k, :],
                                    start=(r == 0),
                                    stop=(r == 7),
                                    tile_position=(0, 32 * c),
                                    skip_group_check=True,
                                )

                    pend.append(pvtgt)

                def pvsrc_nb2(nb2, us=us, mb=mb):
                    ws = wspool.tile([128, NB2], F32, tag="ws", name="ws")
                    for half in range(2):
                        for c in range(4):
                            nc.tensor.matmul(
                                ws[32 * c : 32 * c + 4][
                                    :, half * 512 : (half + 1) * 512
                                ],
                                lhsT=vt[:, (mb * 4 + c) * 4 :][:, :4],
                                rhs=us[c][:, nb2 * NB2 + half * 512 :][:, :512],
                                start=True,
                                stop=True,
                                tile_position=(0, 32 * c),
                                skip_group_check=True,
                            )
                    acc = ws_acc[:, nb2 * NB2 : (nb2 + 1) * NB2]
                    if mb == 0:
                        nc.vector.tensor_copy(acc, ws)
                    else:
                        nc.vector.tensor_add(acc, acc, ws)
                    if mb == NMB - 1:
                        # sync ring is done with transposes by now
                        for c in range(4):
                            nc.sync.dma_start(
                                out=wsrc_d[nb2, 4 * c : 4 * c + 4, :],
                                in_=acc[32 * c : 32 * c + 4, :],
                            )

                def wt_retire(wt=wt, mb=mb):
                    wt_sb = osb.tile([128, 512], F32, tag="wt_sb", name="wt_sb")
                    nc.vector.tensor_copy(wt_sb, wt)
                    ring = nc.sync if mb == NMB - 1 else nc.gpsimd
                    for c in range(4):
                        ring.dma_start(
                            out=wtgt_d[mb, 4 * c : 4 * c + 4, :],
                            in_=wt_sb[32 * c : 32 * c + 4, :],
                        )

                if mb < NMB - 1:
                    for _nb2 in range(NNB2):
                        pend.append(lambda nb2=_nb2, f=pvsrc_nb2: f(nb2))
                    pend.append(wt_retire)
                else:
                    # tail: PV-src is ready NOW (only needs u's); the last
                    # pvtgt's wait on their xbar transposes. Run PV-src
                    # first so the PE isn't idle, then the pvtgt backlog.
                    tail_pv = [t for t in pend]
                    pend.clear()
                    for _nb2 in range(NNB2):
                        pvsrc_nb2(_nb2)
                    for t in tail_pv:
                        t()
                    wt_retire()
            flush(0)

    nc.compile()
    return nc


def _prep_inputs(src, tgt, src_f, tgt_f):
    src = np.ascontiguousarray(src, dtype=np.float32)
    tgt = np.ascontiguousarray(tgt, dtype=np.float32)
    src_f = np.ascontiguousarray(src_f, dtype=np.float32)
    tgt_f = np.ascontiguousarray(tgt_f, dtype=np.float32)

    in_maps = []
    for core in range(NCORES):
        b, h = divmod(core, 2)
        msl = slice(h * MH, (h + 1) * MH)
        L = tgt_f[b][:, msl]  # [64, MH]
        R = src_f[b]  # [64, N]
        aa = (R * R).sum(axis=0)  # [N]
        bb = (L * L).sum(axis=0)  # [MH]
        s_n = (-0.5 * aa).astype(np.float16)  # fp16 shift row
        r_n = s_n.astype(np.float32) + 0.5 * aa  # rounding residual
        lhs = np.vstack([L.astype(np.float16), np.ones((1, MH), np.float16)])
        rhs = np.vstack([R.astype(np.float16), s_n[None, :]])
        bias = np.ascontiguousarray(
            (C0 - bb).reshape(NMC, 128).T.astype(np.float32)
        )  # [128, 16]
        # vt: col block mc = [tgt|1] rows of that m-chunk
        vtgt = np.concatenate([tgt[b][msl], np.ones((MH, 1), np.float32)], 1)
        vt = np.ascontiguousarray(
            vtgt.reshape(NMC, 128, 4).transpose(1, 0, 2).reshape(128, NMC * 4)
        ).astype(np.float16)
        # vs: xbar-permuted [src|1]*exp(-2 r_n): slab k, row p <-> n=k*128+p
        vsrc = np.concatenate([src[b], np.ones((N, 1), np.float32)], 1)
        vsrc = vsrc * np.exp(-2.0 * r_n)[:, None]
        vs = np.ascontiguousarray(
            vsrc.reshape(32, 128, 4).transpose(1, 0, 2).reshape(128, 128)
        )
        in_maps.append(
            {
                "lhs": np.ascontiguousarray(lhs),
                "rhs": np.ascontiguousarray(rhs),
                "bias": bias,
                "vt": vt,
                "vs": vs.astype(NPBF),
            }
        )
    return in_maps


def _postprocess(results):
    src_corr = np.zeros((B, N, 3), np.float32)
    tgt_corr = np.zeros((B, N, 3), np.float32)
    for b in range(B):
        wsum = None
        for h in range(2):
            w = results[2 * b + h]["wsrc"].astype(np.float64)  # [nb2,16,1024]
            w = w.reshape(NNB2, 4, 4, NB2).sum(axis=1)  # [nb2,4,1024]
            w = w.transpose(0, 2, 1).reshape(N, 4)  # [n, 4]
            wsum = w if wsum is None else wsum + w
        src_corr[b] = (wsum[:, :3] / wsum[:, 3:4]).astype(np.float32)
        for h in range(2):
            w = results[2 * b + h]["wtgt"].astype(np.float64)  # [mb,16,512]
            w = w.reshape(NMB, 4, 4, 512).sum(axis=1)  # [mb, 4, 512]
            w = w.transpose(0, 2, 1).reshape(MH, 4)  # [m', 4]
            tgt_corr[b, h * MH : (h + 1) * MH] = (w[:, :3] / w[:, 3:4]).astype(
                np.float32
            )
    return src_corr, tgt_corr


def run(inputs, trace=False, **kw):
    global _PROG
    if _PROG is None:
        _PROG = _build()
    in_maps = _prep_inputs(
        inputs["src"], inputs["tgt"], inputs["src_f"], inputs["tgt_f"]
    )
    bkr = run_bass_kernel_spmd(
        _PROG, in_maps, core_ids=list(range(NCORES)), trace=trace, **kw
    )
    return _postprocess(bkr.results), bkr


def kernel(**inputs):
    out, _ = run(inputs)
    return out
